# revision 40
# baseline (speedup 1.0000x reference)
"""Trainium2 Bass kernel for HeatmapMaxDetBlock (argmax + local refinement).

Computes, for x[B, C, H, W]:
    scores = max over (H*W); idx = argmax; px = idx % W, py = idx // W (masked
    by score > 0); quarter-pixel refinement by sign of neighbor differences.
Returns [B, C, 3] = (px, py, scores).

Strategy (pure data parallel over 8 NeuronCores, batch-sharded):
  Layout: NSEG=128 segments of SEGW=384 per heatmap row; SBUF partition p =
  segment index, M[p, r] = max of (row r, segment p).  One DMA chunk covers
  all 128 segments of `mr` consecutive rows, so rows COMPLETE per chunk and
  phase 2 pipelines under the stream in row groups.

  Per chunk: Pool (gpsimd) does a level-1 tt.max fold (384 -> 192), DVE
  finishes the reduce -- this keeps total DVE work well under the stream
  time so the stream stays HBM-bound.

  Per row group (48/64/16 rows + 8 leftover): PE-transpose the M columns
  (rows land on partitions -- no relayout DMA), winner segment via
  max((M == score) * iorev) (first segment on ties = argmax), one indirect
  window gather of [seg-MARG, seg+SEGW+MARG), then fused one-hot selects:
      rstar = max((win == score) * iorev384)        # first peak on ties
      ddx   = sum((iorev384 == rstar) * (win[+1] - win[-1]))
      ddy   = sum((iorev384 == rstar) * (win[+W] - win[-W]))
  Since SEGW = 2W, px/py need no division: with ii = in-segment index,
      g  = (ii >= W);  px = ii - W*g;  py = 2*s* + g      (all exact f32)
  The last groups are tapered (MD8/4/2) so the final exposed chain is tiny.
"""

import sys
from contextlib import ExitStack
from dataclasses import dataclass

import numpy as np

for _p in ("/opt/trn_rl_repo",):
    if _p not in sys.path:
        sys.path.insert(0, _p)

import concourse.bass as bass  # noqa: E402
import concourse.tile as tile  # noqa: E402
from concourse import bacc, mybir  # noqa: E402
from concourse.masks import make_identity  # noqa: E402

F32 = mybir.dt.float32
U32 = mybir.dt.uint32
AX = mybir.AxisListType
OP = mybir.AluOpType


@dataclass(frozen=True)
class Cfg:
    B: int = 64
    C: int = 17
    H: int = 256
    W: int = 192
    ncores: int = 8
    P: int = 128
    FRONT: int = 256
    REAR: int = 512

    @property
    def BP(self):
        return self.B // self.ncores

    @property
    def R(self):  # heatmap rows per core (136)
        return self.BP * self.C

    @property
    def HWm(self):
        return self.H * self.W

    @property
    def NSEG(self):
        return self.P

    @property
    def SEGW(self):  # 384 = 2*W
        return self.HWm // self.NSEG

    @property
    def MARG(self):
        return self.W + 2

    @property
    def WINW(self):  # 772
        return self.SEGW + 2 * self.MARG

    @property
    def SHN(self):
        return self.R * self.HWm

    @property
    def NPAD(self):
        return self.FRONT + self.SHN + self.REAR


CFG = Cfg()

# (row0, nrows) DMA chunks in STREAM order; groups = (row0, nrows, [chunk
# idxs]).  Rows 0..63 (g0) and the 8 leftover rows (g1) stream early so
# their chains hide under the stream; rows 64..127 (g2) stream last with a
# taper so the one exposed chain starts as early as possible.
def _schedule(c: Cfg):
    chunks = [
        (0, 16), (16, 16), (32, 16), (48, 16),      # g0
        (64, 16), (80, 16),                          # g2 head
        (128, 4), (132, 4),                          # g1 (leftover rows)
        (96, 16), (112, 8), (120, 4), (124, 2), (126, 2),  # g2 tail, tapered
    ]
    assert sorted(q for r0, mr in chunks for q in range(r0, r0 + mr)) == list(
        range(c.R)
    )
    groups = [(0, 64, [0, 1, 2, 3]), (128, 8, [6, 7]), (64, 64, [4, 5, 8, 9, 10, 11, 12])]
    for r0, gn, idxs in groups:
        assert sum(chunks[i][1] for i in idxs) == gn
    return chunks, groups


def build_program(cfg: Cfg):
    c = cfg
    assert c.SEGW == 2 * c.W
    assert c.FRONT >= c.MARG and c.REAR >= c.MARG
    chunks, groups = _schedule(c)

    nc = bacc.Bacc(
        "TRN2", target_bir_lowering=False, debug=False, num_devices=c.ncores
    )
    xh = nc.dram_tensor("x", [c.NPAD], F32, kind="ExternalInput").ap()
    rbh = nc.dram_tensor("rowbase", [c.R, 1], F32, kind="ExternalInput").ap()
    io128h = nc.dram_tensor("iorev128", [c.P, c.NSEG], F32, kind="ExternalInput").ap()
    io384h = nc.dram_tensor("iorev384", [c.P, c.SEGW], F32, kind="ExternalInput").ap()
    oh = nc.dram_tensor("out", [c.R, 3], F32, kind="ExternalOutput").ap()

    with ExitStack() as ctx:
        tc = ctx.enter_context(tile.TileContext(nc))
        xpool = ctx.enter_context(tc.tile_pool(name="xp", bufs=3))
        sp = ctx.enter_context(tc.tile_pool(name="sp", bufs=1))
        pp = ctx.enter_context(tc.tile_pool(name="pp", bufs=1, space="PSUM"))

        M = sp.tile([c.P, c.R], F32, tag="M")
        ident = sp.tile([c.P, c.P], F32, tag="ident")
        make_identity(nc, ident[:])
        io128 = sp.tile([c.P, c.NSEG], F32, tag="io128")
        io384 = sp.tile([c.P, c.SEGW], F32, tag="io384")
        rbt = {}
        for gi, (r0, gn, _) in enumerate(groups):
            rbt[gi] = sp.tile([gn, 1], F32, tag=f"rbt{gi}", name=f"rbt{gi}")

        def issue_dma(i, r0, mr):
            xt = xpool.tile([c.P, mr * c.SEGW], F32, tag=f"xt{mr}")
            src = bass.AP(
                xh.tensor,
                c.FRONT + r0 * c.HWm,
                [[c.SEGW, c.P], [c.HWm, mr], [1, c.SEGW]],
            )
            eng = nc.sync if i % 2 == 0 else nc.scalar
            eng.dma_start(out=xt[:].rearrange("p (m u) -> p m u", m=mr), in_=src)
            return xt

        def reduce_chunk(xt, r0, mr):
            v = xt[:].rearrange("p (m u) -> p m u", m=mr)
            nc.vector.reduce_max(out=M[:, r0 : r0 + mr], in_=v, axis=AX.X)

        # ---- phase 2 per row group ------------------------------------------
        def part1(gi):
            r0, gn, _ = groups[gi]
            mtp = pp.tile([gn, c.P], F32, tag=f"mtp{gi}")
            nc.tensor.transpose(out=mtp[:], in_=M[:, r0 : r0 + gn], identity=ident[:])
            MTt = sp.tile([c.P, c.P], F32, tag="MT")
            MT = MTt[0:gn]
            nc.vector.tensor_copy(out=MT, in_=mtp[:])
            SC = sp.tile([gn, 1], F32, tag=f"SC{gi}")
            nc.vector.tensor_reduce(out=SC[:], in_=MT, axis=AX.X, op=OP.max)
            scr = sp.tile([c.P, c.NSEG], F32, tag="scr128")
            srev = sp.tile([gn, 1], F32, tag=f"sr{gi}")
            nc.vector.scalar_tensor_tensor(
                out=scr[0:gn], in0=MT, scalar=SC[:], in1=io128[0:gn],
                op0=OP.is_equal, op1=OP.mult,
            )
            nc.vector.tensor_reduce(
                out=srev[:], in_=scr[0:gn], axis=AX.X, op=OP.max
            )
            # w0 = (NSEG-1-srev)*SEGW + rowbase  (window start, absolute)
            w0 = sp.tile([gn, 1], F32, tag=f"w0{gi}")
            nc.vector.tensor_scalar(
                out=w0[:], in0=srev[:], scalar1=-float(c.SEGW),
                scalar2=float((c.NSEG - 1) * c.SEGW), op0=OP.mult, op1=OP.add,
            )
            nc.vector.tensor_tensor(out=w0[:], in0=w0[:], in1=rbt[gi][:], op=OP.add)
            w0u = sp.tile([gn, 1], U32, tag=f"w0u{gi}")
            nc.vector.tensor_copy(out=w0u[:], in_=w0[:])
            wint = sp.tile([c.P, c.WINW], F32, tag="win")
            win = wint[0:gn]
            nc.gpsimd.indirect_dma_start(
                out=win, out_offset=None, in_=xh[:, None],
                in_offset=bass.IndirectOffsetOnAxis(ap=w0u[:, 0:1], axis=0),
            )
            return MT, SC, srev, win

        def part2(gi, SC, srev, win):
            r0, gn, _ = groups[gi]
            M0 = c.MARG
            mid = win[:, M0 : M0 + c.SEGW]
            scr = sp.tile([c.P, c.SEGW], F32, tag="scr384")
            rstar = sp.tile([gn, 1], F32, tag=f"rs{gi}")
            nc.vector.scalar_tensor_tensor(
                out=scr[0:gn], in0=mid, scalar=SC[:], in1=io384[0:gn],
                op0=OP.is_equal, op1=OP.mult,
            )
            nc.vector.tensor_reduce(
                out=rstar[:], in_=scr[0:gn], axis=AX.X, op=OP.max
            )
            diff = sp.tile([c.P, 2 * c.SEGW], F32, tag="diff")
            nc.gpsimd.tensor_tensor(
                out=diff[0:gn, 0 : c.SEGW],
                in0=win[:, M0 + 1 : M0 + 1 + c.SEGW],
                in1=win[:, M0 - 1 : M0 - 1 + c.SEGW], op=OP.subtract,
            )
            D = sp.tile([gn, 2], F32, tag=f"D{gi}")
            nc.vector.scalar_tensor_tensor(
                out=scr[0:gn], in0=io384[0:gn], scalar=rstar[:],
                in1=diff[0:gn, 0 : c.SEGW],
                op0=OP.is_equal, op1=OP.mult, accum_out=D[:, 0:1],
            )
            nc.gpsimd.tensor_tensor(
                out=diff[0:gn, c.SEGW :],
                in0=win[:, M0 + c.W : M0 + c.W + c.SEGW],
                in1=win[:, M0 - c.W : M0 - c.W + c.SEGW], op=OP.subtract,
            )
            nc.vector.scalar_tensor_tensor(
                out=scr[0:gn], in0=io384[0:gn], scalar=rstar[:],
                in1=diff[0:gn, c.SEGW :],
                op0=OP.is_equal, op1=OP.mult, accum_out=D[:, 1:2],
            )
            # ii = SEGW-1-rstar; g = (ii >= W); px = ii - W*g; py = 2*s* + g
            # where s* = NSEG-1-srev
            O = sp.tile([gn, 3], F32, tag=f"O{gi}")
            ii = sp.tile([gn, 1], F32, tag=f"ii{gi}")
            nc.vector.tensor_scalar(
                out=ii[:], in0=rstar[:], scalar1=-1.0,
                scalar2=float(c.SEGW - 1), op0=OP.mult, op1=OP.add,
            )
            gsel = sp.tile([gn, 1], F32, tag=f"g{gi}")
            nc.vector.tensor_scalar(
                out=gsel[:], in0=ii[:], scalar1=float(c.W), scalar2=None,
                op0=OP.is_ge,
            )
            nc.vector.scalar_tensor_tensor(
                out=O[:, 0:1], in0=gsel[:], scalar=-float(c.W), in1=ii[:],
                op0=OP.mult, op1=OP.add,
            )
            py0 = sp.tile([gn, 1], F32, tag=f"py{gi}")
            nc.vector.tensor_scalar(
                out=py0[:], in0=srev[:], scalar1=-2.0,
                scalar2=float(2 * (c.NSEG - 1)), op0=OP.mult, op1=OP.add,
            )
            nc.vector.tensor_tensor(out=O[:, 1:2], in0=py0[:], in1=gsel[:], op=OP.add)
            mk = sp.tile([gn, 1], F32, tag=f"mk{gi}")
            nc.vector.tensor_scalar(
                out=mk[:], in0=SC[:], scalar1=0.0, scalar2=None, op0=OP.is_gt
            )
            nc.vector.tensor_tensor(
                out=O[:, 0:2], in0=O[:, 0:2],
                in1=mk[:].to_broadcast([gn, 2]), op=OP.mult,
            )
            hi = sp.tile([gn, 2], F32, tag=f"hi{gi}")
            nc.vector.memset(hi[:, 0:1], float(c.W - 1))
            nc.vector.memset(hi[:, 1:2], float(c.H - 1))
            ilo = sp.tile([gn, 2], F32, tag=f"il{gi}")
            nc.vector.tensor_scalar(
                out=ilo[:], in0=O[:, 0:2], scalar1=0.0, scalar2=None, op0=OP.is_gt
            )
            ihi = sp.tile([gn, 2], F32, tag=f"ih{gi}")
            nc.vector.tensor_tensor(out=ihi[:], in0=O[:, 0:2], in1=hi[:], op=OP.is_lt)
            nc.vector.tensor_tensor(out=ilo[:], in0=ilo[:], in1=ihi[:], op=OP.mult)
            intr = sp.tile([gn, 1], F32, tag=f"in{gi}")
            nc.vector.tensor_reduce(out=intr[:], in_=ilo[:], axis=AX.X, op=OP.min)
            DG = sp.tile([gn, 2], F32, tag=f"DG{gi}")
            DL = sp.tile([gn, 2], F32, tag=f"DL{gi}")
            nc.vector.tensor_scalar(
                out=DG[:], in0=D[:], scalar1=0.0, scalar2=0.25,
                op0=OP.is_gt, op1=OP.mult,
            )
            nc.vector.tensor_scalar(
                out=DL[:], in0=D[:], scalar1=0.0, scalar2=0.25,
                op0=OP.is_lt, op1=OP.mult,
            )
            nc.vector.tensor_tensor(out=DG[:], in0=DG[:], in1=DL[:], op=OP.subtract)
            nc.vector.tensor_tensor(
                out=DG[:], in0=DG[:], in1=intr[:].to_broadcast([gn, 2]), op=OP.mult
            )
            nc.vector.tensor_tensor(out=O[:, 0:2], in0=O[:, 0:2], in1=DG[:], op=OP.add)
            nc.vector.tensor_copy(out=O[:, 2:3], in_=SC[:])
            return O

        # ---- emission schedule ----------------------------------------------
        # stream DMAs + constants up front
        xts = []
        for i, (r0, mr) in enumerate(chunks):
            xts.append(issue_dma(i, r0, mr))
        nc.sync.dma_start(out=io128[:], in_=io128h[:])
        nc.scalar.dma_start(out=io384[:], in_=io384h[:])
        for gi, (r0, gn, _) in enumerate(groups):
            eng = nc.sync if gi % 2 == 0 else nc.scalar
            eng.dma_start(out=rbt[gi][:], in_=rbh[r0 : r0 + gn])

        def emit_reduce(i):
            r0, mr = chunks[i]
            reduce_chunk(xts[i], r0, mr)

        # group chains interleaved with the chunk reduces:
        #   part1(g) right after g's last reduce; part2(g) one chunk later
        #   (so the gather is in flight while the next reduce runs).
        last_chunk_of_group = {g[2][-1]: gi for gi, g in enumerate(groups)}
        pending2 = []  # (gi, state...)
        state1 = {}
        outs = {}
        for i in range(len(chunks)):
            emit_reduce(i)
            if pending2:
                gi = pending2.pop(0)
                MT, SC, srev, win = state1[gi]
                outs[gi] = part2(gi, SC, srev, win)
            if i in last_chunk_of_group:
                gi = last_chunk_of_group[i]
                state1[gi] = part1(gi)
                pending2.append(gi)
        while pending2:
            gi = pending2.pop(0)
            MT, SC, srev, win = state1[gi]
            outs[gi] = part2(gi, SC, srev, win)

        # output DMAs at the end (rings are empty post-stream)
        for gi, (r0, gn, _) in enumerate(groups):
            eng = nc.sync if gi % 2 == 0 else nc.scalar
            eng.dma_start(out=oh[r0 : r0 + gn], in_=outs[gi][:])

    nc.compile()
    return nc


def host_constants(cfg: Cfg):
    c = cfg
    r = np.arange(c.R, dtype=np.float64)
    rowbase = (
        (c.FRONT + r * c.HWm - c.MARG).astype(np.float32).reshape(c.R, 1)
    )
    iorev128 = np.tile(
        (c.NSEG - 1 - np.arange(c.NSEG)).astype(np.float32), (c.P, 1)
    )
    iorev384 = np.tile(
        (c.SEGW - 1 - np.arange(c.SEGW)).astype(np.float32), (c.P, 1)
    )
    return rowbase, iorev128, iorev384


def shard_inputs(cfg: Cfg, x: np.ndarray):
    c = cfg
    rowbase, iorev128, iorev384 = host_constants(c)
    in_maps = []
    for k in range(c.ncores):
        shard = np.ascontiguousarray(
            x[k * c.BP : (k + 1) * c.BP], dtype=np.float32
        ).reshape(-1)
        xp = np.zeros(c.NPAD, np.float32)
        xp[c.FRONT : c.FRONT + c.SHN] = shard
        in_maps.append(
            {
                "x": xp,
                "rowbase": rowbase,
                "iorev128": iorev128,
                "iorev384": iorev384,
            }
        )
    return in_maps


def assemble_out(cfg: Cfg, per_core_outs):
    c = cfg
    outs = [o.reshape(c.BP, c.C, 3).astype(np.float32) for o in per_core_outs]
    return np.concatenate(outs, axis=0)


_PROGRAM = None


def _program():
    global _PROGRAM
    if _PROGRAM is None:
        _PROGRAM = build_program(CFG)
    return _PROGRAM


def kernel(x: np.ndarray) -> np.ndarray:
    from concourse.bass_utils import run_bass_kernel_spmd

    c = CFG
    assert x.shape == (c.B, c.C, c.H, c.W), x.shape
    nc = _program()
    in_maps = shard_inputs(c, np.asarray(x))
    res = run_bass_kernel_spmd(nc, in_maps, core_ids=list(range(c.ncores)))
    return assemble_out(c, [res.results[k]["out"] for k in range(c.ncores)])


# revision 44
# speedup vs baseline: 1.2062x; 1.2062x over previous
"""Trainium2 Bass kernel for HeatmapMaxDetBlock (argmax + local refinement).

Computes, for x[B, C, H, W]:
    scores = max over (H*W); idx = argmax; px = idx % W, py = idx // W (masked
    by score > 0); quarter-pixel refinement by sign of neighbor differences.
Returns [B, C, 3] = (px, py, scores).

Strategy (pure data parallel over 8 NeuronCores, batch-sharded):
  phase 1: stream the whole shard through SBUF once; one DVE reduce_max per
           [128, SEGW] tile gives per-(row, segment) maxima.
  phase 2: tiny ops — PE-transpose the maxima, per-row max + winning segment,
           one indirect-DMA window gather per row group, max_index for the
           exact in-segment position, a second indirect gather of the +-W
           neighborhood, then the scalar-ish refinement math.
"""

import sys
from contextlib import ExitStack
from dataclasses import dataclass

import numpy as np

for _p in ("/opt/trn_rl_repo",):
    if _p not in sys.path:
        sys.path.insert(0, _p)

import concourse.bass as bass  # noqa: E402
import concourse.tile as tile  # noqa: E402
from concourse import bacc, mybir  # noqa: E402
from concourse.masks import make_identity  # noqa: E402

F32 = mybir.dt.float32
U32 = mybir.dt.uint32
AX = mybir.AxisListType
OP = mybir.AluOpType


@dataclass(frozen=True)
class Cfg:
    B: int = 64
    C: int = 17
    H: int = 256
    W: int = 192
    ncores: int = 8
    P: int = 128
    NSEG: int = 64
    MD: int = 4  # tile-columns merged per DMA
    FRONT: int = 256
    REAR: int = 512

    @property
    def BP(self):  # batches per core
        return self.B // self.ncores

    @property
    def R(self):  # heatmap rows per core
        return self.BP * self.C

    @property
    def HWm(self):
        return self.H * self.W

    @property
    def SEGW(self):
        return self.HWm // self.NSEG

    @property
    def RPT(self):  # rows per tile
        return self.P // self.NSEG

    @property
    def NT(self):  # tiles per core
        return self.R // self.RPT

    @property
    def MARG(self):
        return self.W + 2

    @property
    def WINW(self):
        return self.SEGW + 2 * self.MARG

    @property
    def NBW(self):
        return 2 * self.W + 1

    @property
    def SHN(self):
        return self.R * self.HWm

    @property
    def NPAD(self):
        return self.FRONT + self.SHN + self.REAR


CFG = Cfg()


def build_program(cfg: Cfg):
    c = cfg
    assert c.P % c.NSEG == 0 and c.R % c.RPT == 0 and c.HWm % c.NSEG == 0
    assert c.R <= c.P or c.R - c.P in range(0, 17), (
        "group B must fit in one 16-partition pad"
    )
    assert c.FRONT >= c.MARG and c.REAR >= c.MARG
    assert 8 <= c.SEGW <= 16384
    assert c.NT % c.MD == 0 and c.NT <= c.P
    GA = min(c.P, c.R)
    assert GA % c.RPT == 0

    nc = bacc.Bacc(
        "TRN2", target_bir_lowering=False, debug=False, num_devices=c.ncores
    )
    xh = nc.dram_tensor("x", [c.NPAD], F32, kind="ExternalInput").ap()
    rbh = nc.dram_tensor("rowbase", [c.NT, c.RPT], F32, kind="ExternalInput").ap()
    irh = nc.dram_tensor("iotarev", [c.NT, c.P], F32, kind="ExternalInput").ap()
    iofh = nc.dram_tensor("iota768", [c.P, c.SEGW], F32, kind="ExternalInput").ap()
    oh = nc.dram_tensor("out", [c.R, 3], F32, kind="ExternalOutput").ap()

    with ExitStack() as ctx:
        tc = ctx.enter_context(tile.TileContext(nc))
        xpool = ctx.enter_context(tc.tile_pool(name="xp", bufs=3))
        sp = ctx.enter_context(tc.tile_pool(name="sp", bufs=1))
        pp = ctx.enter_context(tc.tile_pool(name="pp", bufs=1, space="PSUM"))

        # ---- phase 1: per-(row, segment) maxima ------------------------------
        # DMA tile g: [P, MD*SEGW]; partition p = RPT-row j * NSEG + seg s;
        # free = MD tile-columns (row-groups) of SEGW. One reduce per DMA
        # yields MD columns of M.
        M = sp.tile([c.P, c.P], F32, tag="M")
        nc.vector.memset(M[:], 0.0)
        mds = [4] * 16 + [2, 2]  # tapered so the last reduce is small
        assert sum(mds) == c.NT
        col = 0
        for g, md in enumerate(mds):
            xt = xpool.tile([c.P, md * c.SEGW], F32, tag=f"xt{md}")
            off = c.FRONT + col * c.RPT * c.HWm
            src = bass.AP(
                xh.tensor,
                off,
                [
                    [c.HWm, c.RPT],
                    [c.SEGW, c.NSEG],
                    [c.RPT * c.HWm, md],
                    [1, c.SEGW],
                ],
            )
            eng = nc.sync if g % 2 == 0 else nc.scalar
            eng.dma_start(
                out=xt[:].rearrange("p (m u) -> p m u", m=md), in_=src
            )
            nc.vector.reduce_max(
                out=M[:, col : col + md],
                in_=xt[:].rearrange("p (m u) -> p m u", m=md),
                axis=AX.X,
            )
            col += md

        # ---- phase 2: find row max + winning segment -------------------------
        ident = sp.tile([c.P, c.P], F32, tag="ident")
        make_identity(nc, ident[:])
        mtp = pp.tile([c.P, c.P], F32)
        nc.tensor.transpose(out=mtp[:], in_=M[:], identity=ident[:])
        MT = sp.tile([c.P, c.P], F32, tag="MT")
        nc.vector.tensor_copy(out=MT[:], in_=mtp[:])
        # MT[t, j*NSEG + s] = max of (row RPT*t + j, segment s)
        MT3 = MT[0 : c.NT].rearrange("p (j s) -> p j s", j=c.RPT)

        scores = sp.tile([c.NT, c.RPT], F32, tag="scores")
        nc.vector.reduce_max(out=scores[:], in_=MT3, axis=AX.X)

        irt = sp.tile([c.NT, c.P], F32, tag="irt")
        nc.sync.dma_start(out=irt[:], in_=irh[:])
        iof = sp.tile([c.P, c.SEGW], F32, tag="iof")
        nc.scalar.dma_start(out=iof[:], in_=iofh[:])
        rbt = sp.tile([c.NT, c.RPT], F32, tag="rbt")
        nc.sync.dma_start(out=rbt[:], in_=rbh[:])

        mk16 = sp.tile([c.NT, c.P], F32, tag="mk16")
        mk16_3 = mk16[:].rearrange("p (j s) -> p j s", j=c.RPT)
        nc.vector.tensor_tensor(
            out=mk16_3,
            in0=MT3,
            in1=scores[:, :, None].to_broadcast([c.NT, c.RPT, c.NSEG]),
            op=OP.is_equal,
        )
        nc.vector.tensor_tensor(
            out=mk16_3,
            in0=mk16_3,
            in1=irt[:].rearrange("p (j s) -> p j s", j=c.RPT),
            op=OP.mult,
        )
        srev = sp.tile([c.NT, c.RPT], F32, tag="srev")
        nc.vector.reduce_max(out=srev[:], in_=mk16_3, axis=AX.X)
        # seg_base = (NSEG-1 - srev) * SEGW
        sb = sp.tile([c.NT, c.RPT], F32, tag="sb")
        nc.vector.tensor_scalar(
            out=sb[:],
            in0=srev[:],
            scalar1=-float(c.SEGW),
            scalar2=float((c.NSEG - 1) * c.SEGW),
            op0=OP.mult,
            op1=OP.add,
        )
        # window start (absolute, in padded x): W0 = seg_base + rowbase
        w0 = sp.tile([c.NT, c.RPT], F32, tag="w0")
        nc.vector.tensor_tensor(out=w0[:], in0=sb[:], in1=rbt[:], op=OP.add)

        # pack (w0, score, seg_base) -> [NT, RPT, 3] for the row-major relayout
        P4 = sp.tile([c.NT, c.RPT * 3], F32, tag="P4")
        P43 = P4[:].rearrange("p (j e) -> p j e", e=3)
        for col, src in enumerate((w0, scores, sb)):
            nc.vector.tensor_copy(out=P43[:, :, col : col + 1], in_=src[:, :, None])

        # relayout to rows-on-partitions: RA rows 0..GA-1, RB rows GA..R-1
        RA = sp.tile([GA, 3], F32, tag="RA")
        nta = GA // c.RPT  # tiles covered by group A
        nc.sync.dma_start(out=RA[:], in_=P43[0:nta])
        if c.R > c.P:
            RB = sp.tile([16, 3], F32, tag="RB")
            nc.vector.memset(RB[:], 0.0)
            nc.sync.dma_start(out=RB[0 : c.R - c.P], in_=P43[nta : c.NT])

        # ---- phase 2b/c per row group ---------------------------------------
        def group(Rt, gp, tagp):
            w0u = sp.tile([gp, 1], U32, tag=f"w0u{tagp}")
            nc.vector.tensor_copy(out=w0u[:], in_=Rt[:, 0:1])
            win = sp.tile([gp, c.WINW], F32, tag=f"win{tagp}")
            nc.gpsimd.indirect_dma_start(
                out=win[:],
                out_offset=None,
                in_=xh[:, None],
                in_offset=bass.IndirectOffsetOnAxis(ap=w0u[:, 0:1], axis=0),
            )
            M0 = c.MARG
            diff = sp.tile([gp, 2 * c.SEGW], F32, tag=f"df{tagp}")
            nc.gpsimd.tensor_tensor(
                out=diff[:, 0 : c.SEGW],
                in0=win[:, M0 + 1 : M0 + 1 + c.SEGW],
                in1=win[:, M0 - 1 : M0 - 1 + c.SEGW],
                op=OP.subtract,
            )
            nc.gpsimd.tensor_tensor(
                out=diff[:, c.SEGW : 2 * c.SEGW],
                in0=win[:, M0 + c.W : M0 + c.W + c.SEGW],
                in1=win[:, M0 - c.W : M0 - c.W + c.SEGW],
                op=OP.subtract,
            )
            m8 = sp.tile([gp, 8], F32, tag=f"m8{tagp}")
            nc.vector.tensor_copy(out=m8[:], in_=Rt[:, 1:2].to_broadcast([gp, 8]))
            mi = sp.tile([gp, 8], U32, tag=f"mi{tagp}")
            nc.vector.max_index(
                mi[:], m8[:], win[:, c.MARG : c.MARG + c.SEGW]
            )
            ii = sp.tile([gp, 1], F32, tag=f"ii{tagp}")
            nc.vector.tensor_copy(out=ii[:], in_=mi[:, 0:1])

            # one-hot select of the +-1 / +-W differences at the peak
            scrD = sp.tile([gp, c.SEGW], F32, tag=f"scrD{tagp}")
            D = sp.tile([gp, 2], F32, tag=f"D{tagp}")
            nc.vector.scalar_tensor_tensor(
                out=scrD[:], in0=iof[0:gp], scalar=ii[:],
                in1=diff[:, 0 : c.SEGW],
                op0=OP.is_equal, op1=OP.mult, accum_out=D[:, 0:1],
            )
            nc.vector.scalar_tensor_tensor(
                out=scrD[:], in0=iof[0:gp], scalar=ii[:],
                in1=diff[:, c.SEGW : 2 * c.SEGW],
                op0=OP.is_equal, op1=OP.mult, accum_out=D[:, 1:2],
            )

            # final math: SEGW = 4*W, so py = sb/W + sum_k (ii >= k*W) and
            # px = idxm - W*py -- all exact integer f32, no casts needed
            O = sp.tile([gp, 3], F32, tag=f"O{tagp}")
            idxm = sp.tile([gp, 1], F32, tag=f"idxm{tagp}")
            nc.vector.tensor_tensor(out=idxm[:], in0=Rt[:, 2:3], in1=ii[:], op=OP.add)
            t1 = sp.tile([gp, 1], F32, tag=f"t1{tagp}")
            t2 = sp.tile([gp, 1], F32, tag=f"t2{tagp}")
            lo = sp.tile([gp, 1], F32, tag=f"lo{tagp}")
            nc.vector.tensor_scalar(
                out=t1[:], in0=ii[:], scalar1=float(c.W), scalar2=None, op0=OP.is_ge
            )
            nc.vector.tensor_scalar(
                out=t2[:], in0=ii[:], scalar1=float(2 * c.W), scalar2=None,
                op0=OP.is_ge,
            )
            nc.vector.tensor_scalar(
                out=lo[:], in0=ii[:], scalar1=float(3 * c.W), scalar2=None,
                op0=OP.is_ge,
            )
            nc.vector.tensor_tensor(out=t1[:], in0=t1[:], in1=t2[:], op=OP.add)
            nc.vector.tensor_tensor(out=t1[:], in0=t1[:], in1=lo[:], op=OP.add)
            nc.vector.tensor_scalar(
                out=t2[:], in0=Rt[:, 2:3], scalar1=1.0 / c.W, scalar2=None,
                op0=OP.mult,
            )
            nc.vector.tensor_tensor(out=O[:, 1:2], in0=t2[:], in1=t1[:], op=OP.add)
            nc.vector.scalar_tensor_tensor(
                out=O[:, 0:1], in0=O[:, 1:2], scalar=-float(c.W), in1=idxm[:],
                op0=OP.mult, op1=OP.add,
            )
            mk = sp.tile([gp, 1], F32, tag=f"mk{tagp}")
            nc.vector.tensor_scalar(
                out=mk[:], in0=Rt[:, 1:2], scalar1=0.0, scalar2=None, op0=OP.is_gt
            )
            nc.vector.tensor_tensor(
                out=O[:, 0:2], in0=O[:, 0:2],
                in1=mk[:].to_broadcast([gp, 2]), op=OP.mult,
            )
            # interior = (0 < px < W-1) & (0 < py < H-1)
            hi = sp.tile([gp, 2], F32, tag=f"hi{tagp}")
            nc.vector.memset(hi[:, 0:1], float(c.W - 1))
            nc.vector.memset(hi[:, 1:2], float(c.H - 1))
            ilo = sp.tile([gp, 2], F32, tag=f"ilo{tagp}")
            nc.vector.tensor_scalar(
                out=ilo[:], in0=O[:, 0:2], scalar1=0.0, scalar2=None, op0=OP.is_gt
            )
            ihi = sp.tile([gp, 2], F32, tag=f"ihi{tagp}")
            nc.vector.tensor_tensor(out=ihi[:], in0=O[:, 0:2], in1=hi[:], op=OP.is_lt)
            nc.vector.tensor_tensor(out=ilo[:], in0=ilo[:], in1=ihi[:], op=OP.mult)
            intr = sp.tile([gp, 1], F32, tag=f"intr{tagp}")
            nc.vector.tensor_reduce(out=intr[:], in_=ilo[:], axis=AX.X, op=OP.min)

            # dx, dy = 0.25 * sign(D) * interior
            DG = sp.tile([gp, 2], F32, tag=f"DG{tagp}")
            DL = sp.tile([gp, 2], F32, tag=f"DL{tagp}")
            nc.vector.tensor_scalar(
                out=DG[:], in0=D[:], scalar1=0.0, scalar2=0.25,
                op0=OP.is_gt, op1=OP.mult,
            )
            nc.vector.tensor_scalar(
                out=DL[:], in0=D[:], scalar1=0.0, scalar2=0.25,
                op0=OP.is_lt, op1=OP.mult,
            )
            nc.vector.tensor_tensor(out=DG[:], in0=DG[:], in1=DL[:], op=OP.subtract)
            nc.vector.tensor_tensor(
                out=DG[:], in0=DG[:], in1=intr[:].to_broadcast([gp, 2]), op=OP.mult
            )
            nc.vector.tensor_tensor(out=O[:, 0:2], in0=O[:, 0:2], in1=DG[:], op=OP.add)
            nc.vector.tensor_copy(out=O[:, 2:3], in_=Rt[:, 1:2])
            return O

        OA = group(RA[:], GA, "a")
        nc.sync.dma_start(out=oh[0:GA], in_=OA[:])
        if c.R > c.P:
            OB = group(RB[:], 16, "b")
            nc.sync.dma_start(out=oh[c.P : c.R], in_=OB[0 : c.R - c.P])

    nc.compile()
    return nc


def host_constants(cfg: Cfg):
    c = cfg
    r = np.arange(c.R, dtype=np.float64)
    rowbase = (c.FRONT + r * c.HWm - c.MARG).astype(np.float32).reshape(c.NT, c.RPT)
    s = np.arange(c.NSEG, dtype=np.float64)
    row = np.tile((c.NSEG - 1 - s), c.RPT).astype(np.float32)  # [P]
    iotarev = np.tile(row, (c.NT, 1)).astype(np.float32)
    iota768 = np.tile(np.arange(c.SEGW, dtype=np.float32), (c.P, 1))
    return rowbase, iotarev, iota768


def shard_inputs(cfg: Cfg, x: np.ndarray):
    c = cfg
    rowbase, iotarev, iota768 = host_constants(c)
    in_maps = []
    for k in range(c.ncores):
        shard = np.ascontiguousarray(
            x[k * c.BP : (k + 1) * c.BP], dtype=np.float32
        ).reshape(-1)
        xp = np.zeros(c.NPAD, np.float32)
        xp[c.FRONT : c.FRONT + c.SHN] = shard
        in_maps.append(
            {"x": xp, "rowbase": rowbase, "iotarev": iotarev, "iota768": iota768}
        )
    return in_maps


def assemble_out(cfg: Cfg, per_core_outs):
    c = cfg
    outs = [o.reshape(c.BP, c.C, 3).astype(np.float32) for o in per_core_outs]
    return np.concatenate(outs, axis=0)


_PROGRAM = None


def _program():
    global _PROGRAM
    if _PROGRAM is None:
        _PROGRAM = build_program(CFG)
    return _PROGRAM


def kernel(x: np.ndarray) -> np.ndarray:
    from concourse.bass_utils import run_bass_kernel_spmd

    c = CFG
    assert x.shape == (c.B, c.C, c.H, c.W), x.shape
    nc = _program()
    in_maps = shard_inputs(c, np.asarray(x))
    res = run_bass_kernel_spmd(nc, in_maps, core_ids=list(range(c.ncores)))
    return assemble_out(c, [res.results[k]["out"] for k in range(c.ncores)])

